# revision 21
# baseline (speedup 1.0000x reference)
"""Trainium2 Bass kernel for nn_CyclicPositional.

Reference computation (B=16, D=1024, K=64, N=8192):
    Delta[b,k] = 2*pi*k / lengths[b]            k = 1..K
    feats[b,n,2k]   = cos(n*Delta[b,k]) / sqrt(K)
    feats[b,n,2k+1] = sin(n*Delta[b,k]) / sqrt(K)
    feats masked to n < lengths[b]
    pos_emb = feats @ W.T          (W: [D, 2K])
    returns (pos_emb [B,N,D] f32, mask [B,N] bool)

Strategy: data-parallel over batch, 2 batches per core on 8 cores.
On-device trig is avoided entirely via angle addition: with n = 128*q + r,
    cos(n*Delta) = Cq*cos(r*Delta) - Sq*sin(r*Delta)
    sin(n*Delta) = Sq*cos(r*Delta) + Cq*sin(r*Delta)
so a [128 feats-rows, 128 n-cols] tile of feats^T is
    featsT = A[:,q] * CrT + B[:,q] * SrT     (2 DVE ops)
with tiny per-batch host tables (float64 trig, rounded to f32):
    CrT/SrT [128,128]  - cos/sin(r*Delta_k), row-pair duplicated
    A/B     [128,64]   - per-tile rotation scalars, amp NOT folded (amp folds
                         into W^T instead)
Each feats tile is split hi/lo into bf16 (exact: fd = feats - bf16(feats) is
Sterbenz-exact) and used as the stationary operand of 3 accumulated bf16
matmuls per 512-col chunk of the replicated W^T (Fh@Wh + Fh@Wl + Fl@Wh,
~4e-6 rel rms); PSUM->SBUF copies apply the n<length row mask for free via
activation(Copy, scale=per-partition mask); 512KB DMA out per tile.

Measured on the 8-core axon TRN2 pod: ~215us per core per pass (DMA-out
roofline for the 64MiB/core output is ~170us; PE floor for 6x 512-col bf16
matmuls/tile is ~198us).
"""

import math

import numpy as np

import concourse.bass as bass
import concourse.mybir as mybir
from concourse.tile import TileContext
from concourse.bass_utils import run_bass_kernel_spmd

B = 16
D = 1024
K = 64
N_MAX = 8192
N_CORES = 8
BPC = B // N_CORES          # batches per core = 2
QT = N_MAX // 128           # 128-row tiles per batch = 64
NT = BPC * QT               # tiles per core = 128
AMP = 1.0 / math.sqrt(K)

F32 = mybir.dt.float32

LAST_RESULT = None          # last BassKernelResults, for dev harnesses

# Number of times the device-side tile loop re-runs (For_i). The output is
# rewritten identically each pass, so results are unchanged; bench_util uses
# (wall[R=257] - wall[R=1]) / 256 to time the kernel body without an NTFF
# profiler (the axon NTFF hook is absent in this container). Graded runs
# use 1.
REPEAT = 1

# Matmul numerics: "bf16split" = bf16 hi/lo decomposition, 3 accumulated
# bf16 matmuls per 512-col chunk (~4e-6 rel rms, 1 PE cycle/row, measured
# 215us/core); "fp32" = plain fp32 matmuls (exact, 4 PE cycles/row). The
# other modes are dev diagnostics. Dev harnesses may override these module
# attributes; the graded path uses the defaults.
MM_MODE = "bf16split"


def _legalize_waits(nc, max_waits=1):
    """The walrus in this container rejects instructions carrying >2 sync
    waits (Tile's kernel-tail drain can accumulate more). Hoist excess waits
    onto injected same-engine NOPs placed immediately before."""
    idx = 0
    for fn in nc.m.functions:
        for bb in fn.blocks:
            new = []
            for inst in bb.instructions:
                si = inst.sync_info
                if si is not None and si.on_wait and len(si.on_wait) > max_waits:
                    waits = list(si.on_wait)
                    hoist, keep = waits[:-max_waits], waits[-max_waits:]
                    for cs in range(0, len(hoist), max_waits):
                        chunk = hoist[cs:cs + max_waits]
                        nop = mybir.InstNoOp(
                            name=f"I-waitfix-{idx}", engine=inst.engine,
                            ins=[], outs=[],
                            sync_info=mybir.SyncInfo(on_wait=chunk, on_update=[]),
                        )
                        idx += 1
                        new.append(nop)
                    inst.sync_info = mybir.SyncInfo(
                        on_wait=keep, on_update=list(si.on_update or []))
                new.append(inst)
            bb.instructions = new
    return idx


def _build_program(repeat=1, mm_mode="bf16split"):
    nc = bass.Bass()
    wt = nc.dram_tensor("wt", [128, D], F32, kind="ExternalInput")
    crt = nc.dram_tensor("crt", [128, BPC * 128], F32, kind="ExternalInput")
    srt = nc.dram_tensor("srt", [128, BPC * 128], F32, kind="ExternalInput")
    ab = nc.dram_tensor("ab", [128, BPC * 2 * QT], F32, kind="ExternalInput")
    msk = nc.dram_tensor("msk", [128, BPC * QT], F32, kind="ExternalInput")
    out = nc.dram_tensor("out", [BPC * N_MAX, D], F32, kind="ExternalOutput")

    BF16 = mybir.dt.bfloat16
    mult = mybir.AluOpType.mult
    add = mybir.AluOpType.add
    sub = mybir.AluOpType.subtract
    Copy = mybir.ActivationFunctionType.Copy

    with TileContext(nc) as tc:
        with (
            tc.tile_pool(name="const", bufs=1) as cpool,
            tc.tile_pool(name="feats", bufs=10) as fpool,
            tc.tile_pool(name="psum", bufs=8, space="PSUM") as ppool,
            tc.tile_pool(name="psum2", bufs=4, space="PSUM") as ppool2,
            tc.tile_pool(name="outp", bufs=8) as opool,
        ):
            wt_s = cpool.tile([128, D], F32, tag="wt")
            crt_s = cpool.tile([128, BPC * 128], F32, tag="crt")
            srt_s = cpool.tile([128, BPC * 128], F32, tag="srt")
            ab_s = cpool.tile([128, BPC * 2 * QT], F32, tag="ab")
            msk_s = cpool.tile([128, BPC * QT], F32, tag="msk")
            nc.sync.dma_start(out=crt_s[:, :], in_=crt[:, :])
            nc.sync.dma_start(out=srt_s[:, :], in_=srt[:, :])
            nc.sync.dma_start(out=ab_s[:, :], in_=ab[:, :])
            nc.sync.dma_start(out=msk_s[:, :], in_=msk[:, :])
            nc.sync.dma_start(out=wt_s[:, :], in_=wt[:, :])

            if mm_mode in ("bf16split", "nocopy", "nodma", "bal", "v2", "v3", "v4"):
                wh_s = cpool.tile([128, D], BF16, tag="wh")
                wl_s = cpool.tile([128, D], BF16, tag="wl")
                wd_s = cpool.tile([128, D], F32, tag="wd")
                nc.vector.tensor_copy(out=wh_s[:, :], in_=wt_s[:, :])
                nc.vector.tensor_tensor(
                    out=wd_s[:, :], in0=wt_s[:, :], in1=wh_s[:, :], op=sub)
                nc.vector.tensor_copy(out=wl_s[:, :], in_=wd_s[:, :])

            if mm_mode in ("dmaonly", "nocopy"):
                zt = cpool.tile([128, D], F32, tag="zt")
                nc.vector.memset(zt[:, :], 0.0)

            def body_v2(_iv=None):
                # 3 full-width (1024-col) matmuls into a 2-bank PSUM group,
                # one masked ACT copy, gpsimd handles the bf16 hi/lo split.
                for b in range(BPC):
                    cr_b = crt_s[:, b * 128:(b + 1) * 128]
                    sr_b = srt_s[:, b * 128:(b + 1) * 128]
                    for q in range(QT):
                        t = b * QT + q
                        a_ap = ab_s[:, b * 2 * QT + q: b * 2 * QT + q + 1]
                        b_ap = ab_s[:, b * 2 * QT + QT + q: b * 2 * QT + QT + q + 1]
                        m_ap = msk_s[:, t:t + 1]

                        tmp = fpool.tile([128, 128], F32, tag="tmp")
                        feats = fpool.tile([128, 128], F32, tag="feats")
                        nc.vector.tensor_scalar_mul(tmp[:, :], sr_b, b_ap)
                        nc.vector.scalar_tensor_tensor(
                            feats[:, :], cr_b, a_ap, tmp[:, :], op0=mult, op1=add)

                        fh = fpool.tile([128, 128], BF16, tag="fh")
                        fl = fpool.tile([128, 128], BF16, tag="fl")
                        fd = fpool.tile([128, 128], F32, tag="fd")
                        nc.gpsimd.tensor_copy(out=fh[:, :], in_=feats[:, :])
                        nc.vector.tensor_tensor(
                            out=fd[:, :], in0=feats[:, :], in1=fh[:, :], op=sub)
                        nc.gpsimd.tensor_copy(out=fl[:, :], in_=fd[:, :])

                        p = ppool2.tile([128, D], F32, tag="p2")
                        nc.tensor.matmul(p[:, :], fh[:, :], wh_s[:, :],
                                         start=True, stop=False)
                        nc.tensor.matmul(p[:, :], fh[:, :], wl_s[:, :],
                                         start=False, stop=False)
                        nc.tensor.matmul(p[:, :], fl[:, :], wh_s[:, :],
                                         start=False, stop=True)

                        ot = opool.tile([128, D], F32, tag="ot")
                        nc.scalar.activation(ot[:, :], p[:, :], Copy,
                                             bias=0.0, scale=m_ap)
                        nc.sync.dma_start(
                            out=out[t * 128:(t + 1) * 128, :], in_=ot[:, :])

            def body_v3(_iv=None):
                # Engine-balanced: ACT does the SrT*B multiply and the j0
                # PSUM drain, DVE does the combine/sub and the j1 drain,
                # GpSimd does the two f32->bf16 rounding copies, PE runs the
                # 6 bf16 matmuls (the per-tile floor).
                for b in range(BPC):
                    cr_b = crt_s[:, b * 128:(b + 1) * 128]
                    sr_b = srt_s[:, b * 128:(b + 1) * 128]
                    for q in range(QT):
                        t = b * QT + q
                        a_ap = ab_s[:, b * 2 * QT + q: b * 2 * QT + q + 1]
                        b_ap = ab_s[:, b * 2 * QT + QT + q: b * 2 * QT + QT + q + 1]
                        m_ap = msk_s[:, t:t + 1]

                        tmp = fpool.tile([128, 128], F32, tag="tmp")
                        feats = fpool.tile([128, 128], F32, tag="feats")
                        nc.scalar.activation(tmp[:, :], sr_b, Copy,
                                             bias=0.0, scale=b_ap)
                        nc.vector.scalar_tensor_tensor(
                            feats[:, :], cr_b, a_ap, tmp[:, :], op0=mult, op1=add)

                        fh = fpool.tile([128, 128], BF16, tag="fh")
                        fl = fpool.tile([128, 128], BF16, tag="fl")
                        fd = fpool.tile([128, 128], F32, tag="fd")
                        nc.gpsimd.tensor_copy(out=fh[:, :], in_=feats[:, :])
                        nc.vector.tensor_tensor(
                            out=fd[:, :], in0=feats[:, :], in1=fh[:, :], op=sub)
                        nc.gpsimd.tensor_copy(out=fl[:, :], in_=fd[:, :])

                        ot = opool.tile([128, D], F32, tag="ot")
                        for j in range(2):
                            p = ppool.tile([128, 512], F32, tag="p")
                            wslice = slice(512 * j, 512 * (j + 1))
                            nc.tensor.matmul(p[:, :], fh[:, :], wh_s[:, wslice],
                                             start=True, stop=False)
                            nc.tensor.matmul(p[:, :], fh[:, :], wl_s[:, wslice],
                                             start=False, stop=False)
                            nc.tensor.matmul(p[:, :], fl[:, :], wh_s[:, wslice],
                                             start=False, stop=True)
                            if j == 0:
                                nc.scalar.activation(ot[:, wslice], p[:, :],
                                                     Copy, bias=0.0, scale=m_ap)
                            else:
                                nc.vector.tensor_scalar_mul(
                                    ot[:, wslice], p[:, :], m_ap)
                        nc.sync.dma_start(
                            out=out[t * 128:(t + 1) * 128, :], in_=ot[:, :])

            def body_v4(_iv=None):
                # Like bf16split but the 3-matmul groups land in a 2-bank
                # [128,1024] PSUM tile (512-col matmuls into each half) so
                # the masked drain is ONE 1024-wide ACT op; ACT also does the
                # fh rounding copy to relieve DVE. GpSimd stays idle (slower
                # than modeled on real silicon).
                for b in range(BPC):
                    cr_b = crt_s[:, b * 128:(b + 1) * 128]
                    sr_b = srt_s[:, b * 128:(b + 1) * 128]
                    for q in range(QT):
                        t = b * QT + q
                        a_ap = ab_s[:, b * 2 * QT + q: b * 2 * QT + q + 1]
                        b_ap = ab_s[:, b * 2 * QT + QT + q: b * 2 * QT + QT + q + 1]
                        m_ap = msk_s[:, t:t + 1]

                        tmp = fpool.tile([128, 128], F32, tag="tmp")
                        feats = fpool.tile([128, 128], F32, tag="feats")
                        nc.vector.tensor_scalar_mul(tmp[:, :], sr_b, b_ap)
                        nc.vector.scalar_tensor_tensor(
                            feats[:, :], cr_b, a_ap, tmp[:, :], op0=mult, op1=add)

                        fh = fpool.tile([128, 128], BF16, tag="fh")
                        fl = fpool.tile([128, 128], BF16, tag="fl")
                        fd = fpool.tile([128, 128], F32, tag="fd")
                        nc.scalar.activation(fh[:, :], feats[:, :], Copy,
                                             bias=0.0, scale=1.0)
                        nc.vector.tensor_tensor(
                            out=fd[:, :], in0=feats[:, :], in1=fh[:, :], op=sub)
                        nc.vector.tensor_copy(out=fl[:, :], in_=fd[:, :])

                        p = ppool2.tile([128, D], F32, tag="p2")
                        for j in range(2):
                            ps_ap = p[:, 512 * j:512 * (j + 1)]
                            wslice = slice(512 * j, 512 * (j + 1))
                            nc.tensor.matmul(ps_ap, fh[:, :], wh_s[:, wslice],
                                             start=True, stop=False)
                            nc.tensor.matmul(ps_ap, fh[:, :], wl_s[:, wslice],
                                             start=False, stop=False)
                            nc.tensor.matmul(ps_ap, fl[:, :], wh_s[:, wslice],
                                             start=False, stop=True)

                        ot = opool.tile([128, D], F32, tag="ot")
                        nc.scalar.activation(ot[:, :], p[:, :], Copy,
                                             bias=0.0, scale=m_ap)
                        nc.sync.dma_start(
                            out=out[t * 128:(t + 1) * 128, :], in_=ot[:, :])

            def body(_iv=None):
                if mm_mode == "v4":
                    body_v4(_iv)
                    return
                if mm_mode == "v3":
                    body_v3(_iv)
                    return
                if mm_mode == "v2":
                    body_v2(_iv)
                    return
                if mm_mode == "dmaonly":
                    for t in range(NT):
                        nc.sync.dma_start(
                            out=out[t * 128:(t + 1) * 128, :], in_=zt[:, :])
                    return
                for b in range(BPC):
                    cr_b = crt_s[:, b * 128:(b + 1) * 128]
                    sr_b = srt_s[:, b * 128:(b + 1) * 128]
                    for q in range(QT):
                        t = b * QT + q
                        a_ap = ab_s[:, b * 2 * QT + q: b * 2 * QT + q + 1]
                        b_ap = ab_s[:, b * 2 * QT + QT + q: b * 2 * QT + QT + q + 1]
                        m_ap = msk_s[:, t:t + 1]

                        tmp = fpool.tile([128, 128], F32, tag="tmp")
                        feats = fpool.tile([128, 128], F32, tag="feats")
                        nc.vector.tensor_scalar_mul(tmp[:, :], sr_b, b_ap)
                        nc.vector.scalar_tensor_tensor(
                            feats[:, :], cr_b, a_ap, tmp[:, :], op0=mult, op1=add)

                        if mm_mode in ("bf16split", "nocopy", "nodma", "bal"):
                            eng = nc.gpsimd if mm_mode == "bal" else nc.vector
                            fh = fpool.tile([128, 128], BF16, tag="fh")
                            fl = fpool.tile([128, 128], BF16, tag="fl")
                            fd = fpool.tile([128, 128], F32, tag="fd")
                            eng.tensor_copy(out=fh[:, :], in_=feats[:, :])
                            eng.tensor_tensor(
                                out=fd[:, :], in0=feats[:, :], in1=fh[:, :], op=sub)
                            eng.tensor_copy(out=fl[:, :], in_=fd[:, :])

                        ot = zt if mm_mode == "nocopy" else opool.tile(
                            [128, D], F32, tag="ot")
                        for j in range(2):
                            p = ppool.tile([128, 512], F32, tag="p")
                            wslice = slice(512 * j, 512 * (j + 1))
                            if mm_mode in ("bf16split", "nocopy", "nodma", "bal"):
                                nc.tensor.matmul(
                                    p[:, :], fh[:, :], wh_s[:, wslice],
                                    start=True, stop=False)
                                nc.tensor.matmul(
                                    p[:, :], fh[:, :], wl_s[:, wslice],
                                    start=False, stop=False)
                                nc.tensor.matmul(
                                    p[:, :], fl[:, :], wh_s[:, wslice],
                                    start=False, stop=True)
                            else:
                                nc.tensor.matmul(
                                    p[:, :], feats[:, :], wt_s[:, wslice],
                                    start=True, stop=True)
                            if mm_mode == "bal" and j == 1:
                                nc.vector.tensor_scalar_mul(
                                    ot[:, wslice], p[:, :], m_ap)
                            elif mm_mode != "nocopy":
                                nc.scalar.activation(
                                    ot[:, wslice], p[:, :], Copy,
                                    bias=0.0, scale=m_ap)
                        if mm_mode != "nodma":
                            src_t = zt if mm_mode == "nocopy" else ot
                            nc.sync.dma_start(
                                out=out[t * 128:(t + 1) * 128, :], in_=src_t[:, :])

            if repeat == 1:
                body()
            else:
                with tc.For_i(0, repeat, 1):
                    body()

    _legalize_waits(nc)
    return nc


def _host_tables(lengths):
    """Per-batch f32 tables: CrT/SrT [128,128], A/B [128,QT], M [128,QT]."""
    k = np.arange(1, K + 1, dtype=np.float64)
    crt = np.empty((B, 128, 128), np.float32)
    srt = np.empty((B, 128, 128), np.float32)
    ab = np.empty((B, 128, 2 * QT), np.float32)
    msk = np.empty((B, 128, QT), np.float32)
    r = np.arange(128, dtype=np.float64)
    q128 = 128.0 * np.arange(QT, dtype=np.float64)
    p = np.arange(128)
    for bi in range(B):
        L = float(lengths[bi])
        d = 2.0 * math.pi * k / L                      # [K]
        cr = np.cos(d[:, None] * r[None, :])           # [K,128]
        sr = np.sin(d[:, None] * r[None, :])
        crt[bi] = np.repeat(cr, 2, axis=0)
        srt[bi] = np.repeat(sr, 2, axis=0)
        cq = np.cos(d[:, None] * q128[None, :])        # [K,QT]
        sq = np.sin(d[:, None] * q128[None, :])
        a = np.empty((128, QT), np.float64)
        bb = np.empty((128, QT), np.float64)
        a[0::2] = cq
        a[1::2] = sq
        bb[0::2] = -sq
        bb[1::2] = cq
        ab[bi, :, :QT] = a
        ab[bi, :, QT:] = bb
        msk[bi] = ((q128[None, :] + p[:, None]) < L)
    return crt, srt, ab, msk


def _prepare_in_maps(lengths, W):
    wt = np.ascontiguousarray(W.T * AMP).astype(np.float32)   # [128, D]
    crt, srt, ab, msk = _host_tables(lengths)
    in_maps = []
    for c in range(N_CORES):
        b0, b1 = BPC * c, BPC * c + 1
        in_maps.append({
            "wt": wt,
            "crt": np.ascontiguousarray(np.concatenate([crt[b0], crt[b1]], axis=1)),
            "srt": np.ascontiguousarray(np.concatenate([srt[b0], srt[b1]], axis=1)),
            "ab": np.ascontiguousarray(np.concatenate([ab[b0], ab[b1]], axis=1)),
            "msk": np.ascontiguousarray(np.concatenate([msk[b0], msk[b1]], axis=1)),
        })
    return in_maps


def kernel(lengths, W, N_max):
    global LAST_RESULT
    lengths = np.asarray(lengths)
    W = np.asarray(W, dtype=np.float32)
    n_max = int(N_max)
    assert n_max == N_MAX and W.shape == (D, 2 * K) and lengths.shape == (B,)

    in_maps = _prepare_in_maps(lengths, W)
    nc = _build_program(repeat=REPEAT, mm_mode=MM_MODE)

    res = run_bass_kernel_spmd(nc, in_maps, core_ids=list(range(N_CORES)))
    LAST_RESULT = res

    pos_emb = np.empty((B, N_MAX, D), np.float32)
    for c in range(N_CORES):
        pos_emb[BPC * c:BPC * (c + 1)] = res.results[c]["out"].reshape(BPC, N_MAX, D)

    mask = np.arange(N_MAX)[None, :] < np.asarray(lengths).astype(np.int64)[:, None]
    return pos_emb, mask


# revision 27
# speedup vs baseline: 1.4723x; 1.4723x over previous
"""Trainium2 Bass kernel for nn_CyclicPositional.

Reference computation (B=16, D=1024, K=64, N=8192):
    Delta[b,k] = 2*pi*k / lengths[b]            k = 1..K
    feats[b,n,2k]   = cos(n*Delta[b,k]) / sqrt(K)
    feats[b,n,2k+1] = sin(n*Delta[b,k]) / sqrt(K)
    feats masked to n < lengths[b]
    pos_emb = feats @ W.T          (W: [D, 2K])
    returns (pos_emb [B,N,D] f32, mask [B,N] bool)

Strategy: data-parallel over batch, 2 batches per core on 8 cores.
On-device trig is avoided entirely via angle addition: with n = 128*q + r,
    cos(n*Delta) = Cq*cos(r*Delta) - Sq*sin(r*Delta)
    sin(n*Delta) = Sq*cos(r*Delta) + Cq*sin(r*Delta)
so a [128 feats-rows, 128 n-cols] tile of feats^T is
    featsT = A[:,q] * CrT + B[:,q] * SrT     (2 DVE ops)
with tiny per-batch host tables (float64 trig, rounded to f32):
    CrT/SrT [128,128]  - cos/sin(r*Delta_k), row-pair duplicated
    A/B     [128,64]   - per-tile rotation scalars, amp NOT folded (amp folds
                         into W^T instead)
Each feats tile is split hi/lo into bf16 (exact: fd = feats - bf16(feats) is
Sterbenz-exact) and used as the stationary operand of 3 accumulated bf16
matmuls per 512-col chunk of the replicated W^T (Fh@Wh + Fh@Wl + Fl@Wh,
~4e-6 rel rms); PSUM->SBUF copies apply the n<length row mask for free via
activation(Copy, scale=per-partition mask); 512KB DMA out per tile.

Work distribution (mode "v5"): the runtime zero-fills output buffers, so
fully-masked tiles (n >= length) need neither compute nor DMA. kernel()
compiles after seeing `lengths`, so the ~sum(ceil(L_b/128)) unmasked tiles
are distributed evenly as G slots per core (per-slot r-table streams keep
every access pattern static); each core writes slot-ordered rows and the
host gathers them back into [B, N, D]. Within-session A/B on the 8-core
axon TRN2 pod: v5 ~150us/core/pass vs ~202us for the uniform 128-tile
split (DMA-out roofline for 64MiB/core is ~170us; v5 writes ~50MiB).
"""

import math

import numpy as np

import concourse.bass as bass
import concourse.mybir as mybir
from concourse.tile import TileContext
from concourse.bass_utils import run_bass_kernel_spmd

B = 16
D = 1024
K = 64
N_MAX = 8192
N_CORES = 8
BPC = B // N_CORES          # batches per core = 2
QT = N_MAX // 128           # 128-row tiles per batch = 64
NT = BPC * QT               # tiles per core = 128
AMP = 1.0 / math.sqrt(K)

F32 = mybir.dt.float32

LAST_RESULT = None          # last BassKernelResults, for dev harnesses

# Number of times the device-side tile loop re-runs (For_i). The output is
# rewritten identically each pass, so results are unchanged; bench_util uses
# (wall[R=257] - wall[R=1]) / 256 to time the kernel body without an NTFF
# profiler (the axon NTFF hook is absent in this container). Graded runs
# use 1.
REPEAT = 1

# Matmul numerics: "bf16split" = bf16 hi/lo decomposition, 3 accumulated
# bf16 matmuls per 512-col chunk (~4e-6 rel rms, 1 PE cycle/row, measured
# 215us/core); "fp32" = plain fp32 matmuls (exact, 4 PE cycles/row). The
# other modes are dev diagnostics. Dev harnesses may override these module
# attributes; the graded path uses the defaults.
MM_MODE = "v5"

# Tile-pool depths (working feats tiles / staged output tiles). 6/6 is the
# best-measured configuration; dev harnesses may override.
FPOOL_BUFS = 6
OPOOL_BUFS = 6


def _legalize_waits(nc, max_waits=1):
    """The walrus in this container rejects instructions carrying >2 sync
    waits (Tile's kernel-tail drain can accumulate more). Hoist excess waits
    onto injected same-engine NOPs placed immediately before."""
    idx = 0
    for fn in nc.m.functions:
        for bb in fn.blocks:
            new = []
            for inst in bb.instructions:
                si = inst.sync_info
                if si is not None and si.on_wait and len(si.on_wait) > max_waits:
                    waits = list(si.on_wait)
                    hoist, keep = waits[:-max_waits], waits[-max_waits:]
                    for cs in range(0, len(hoist), max_waits):
                        chunk = hoist[cs:cs + max_waits]
                        nop = mybir.InstNoOp(
                            name=f"I-waitfix-{idx}", engine=inst.engine,
                            ins=[], outs=[],
                            sync_info=mybir.SyncInfo(on_wait=chunk, on_update=[]),
                        )
                        idx += 1
                        new.append(nop)
                    inst.sync_info = mybir.SyncInfo(
                        on_wait=keep, on_update=list(si.on_update or []))
                new.append(inst)
            bb.instructions = new
    return idx


def _build_program(repeat=1, mm_mode="bf16split", fbufs=None, obufs=None,
                   g_slots=None):
    nc = bass.Bass()
    wt = nc.dram_tensor("wt", [128, D], F32, kind="ExternalInput")
    if mm_mode == "v5":
        crt = nc.dram_tensor("crt", [128, g_slots * 128], F32, kind="ExternalInput")
        srt = nc.dram_tensor("srt", [128, g_slots * 128], F32, kind="ExternalInput")
        ab = nc.dram_tensor("ab", [128, 2 * g_slots], F32, kind="ExternalInput")
        msk = nc.dram_tensor("msk", [128, g_slots], F32, kind="ExternalInput")
        out = nc.dram_tensor("out", [g_slots * 128, D], F32, kind="ExternalOutput")
    else:
        crt = nc.dram_tensor("crt", [128, BPC * 128], F32, kind="ExternalInput")
        srt = nc.dram_tensor("srt", [128, BPC * 128], F32, kind="ExternalInput")
        ab = nc.dram_tensor("ab", [128, BPC * 2 * QT], F32, kind="ExternalInput")
        msk = nc.dram_tensor("msk", [128, BPC * QT], F32, kind="ExternalInput")
        out = nc.dram_tensor("out", [BPC * N_MAX, D], F32, kind="ExternalOutput")

    BF16 = mybir.dt.bfloat16
    mult = mybir.AluOpType.mult
    add = mybir.AluOpType.add
    sub = mybir.AluOpType.subtract
    Copy = mybir.ActivationFunctionType.Copy

    with TileContext(nc) as tc:
        with (
            tc.tile_pool(name="const", bufs=1) as cpool,
            tc.tile_pool(name="feats", bufs=fbufs or FPOOL_BUFS) as fpool,
            tc.tile_pool(name="psum", bufs=8, space="PSUM") as ppool,
            tc.tile_pool(name="psum2", bufs=4, space="PSUM") as ppool2,
            tc.tile_pool(name="outp", bufs=obufs or OPOOL_BUFS) as opool,
        ):
            wt_s = cpool.tile([128, D], F32, tag="wt")
            if mm_mode == "v5":
                crts_s = cpool.tile([128, g_slots * 128], F32, tag="crts")
                srts_s = cpool.tile([128, g_slots * 128], F32, tag="srts")
                ab_s = cpool.tile([128, 2 * g_slots], F32, tag="ab")
                msk_s = cpool.tile([128, g_slots], F32, tag="msk")
                nc.sync.dma_start(out=wt_s[:, :], in_=wt[:, :])
                nc.sync.dma_start(out=ab_s[:, :], in_=ab[:, :])
                nc.sync.dma_start(out=msk_s[:, :], in_=msk[:, :])
                for c0 in range(0, g_slots, 16):
                    c1 = min(c0 + 16, g_slots)
                    nc.sync.dma_start(out=crts_s[:, c0 * 128:c1 * 128],
                                      in_=crt[:, c0 * 128:c1 * 128])
                    nc.sync.dma_start(out=srts_s[:, c0 * 128:c1 * 128],
                                      in_=srt[:, c0 * 128:c1 * 128])
            else:
                crt_s = cpool.tile([128, BPC * 128], F32, tag="crt")
                srt_s = cpool.tile([128, BPC * 128], F32, tag="srt")
                ab_s = cpool.tile([128, BPC * 2 * QT], F32, tag="ab")
                msk_s = cpool.tile([128, BPC * QT], F32, tag="msk")
                nc.sync.dma_start(out=wt_s[:, :], in_=wt[:, :])
                nc.sync.dma_start(out=crt_s[:, :], in_=crt[:, :])
                nc.sync.dma_start(out=srt_s[:, :], in_=srt[:, :])
                nc.sync.dma_start(out=ab_s[:, :], in_=ab[:, :])
                nc.sync.dma_start(out=msk_s[:, :], in_=msk[:, :])

            if mm_mode in ("bf16split", "nocopy", "nodma", "bal", "v2", "v3", "v4", "v5"):
                wh_s = cpool.tile([128, D], BF16, tag="wh")
                wl_s = cpool.tile([128, D], BF16, tag="wl")
                wd_s = cpool.tile([128, D], F32, tag="wd")
                nc.vector.tensor_copy(out=wh_s[:, :], in_=wt_s[:, :])
                nc.vector.tensor_tensor(
                    out=wd_s[:, :], in0=wt_s[:, :], in1=wh_s[:, :], op=sub)
                nc.vector.tensor_copy(out=wl_s[:, :], in_=wd_s[:, :])

            if mm_mode in ("dmaonly", "nocopy"):
                zt = cpool.tile([128, D], F32, tag="zt")
                nc.vector.memset(zt[:, :], 0.0)

            def body_v2(_iv=None):
                # 3 full-width (1024-col) matmuls into a 2-bank PSUM group,
                # one masked ACT copy, gpsimd handles the bf16 hi/lo split.
                for b in range(BPC):
                    cr_b = crt_s[:, b * 128:(b + 1) * 128]
                    sr_b = srt_s[:, b * 128:(b + 1) * 128]
                    for q in range(QT):
                        t = b * QT + q
                        a_ap = ab_s[:, b * 2 * QT + q: b * 2 * QT + q + 1]
                        b_ap = ab_s[:, b * 2 * QT + QT + q: b * 2 * QT + QT + q + 1]
                        m_ap = msk_s[:, t:t + 1]

                        tmp = fpool.tile([128, 128], F32, tag="tmp")
                        feats = fpool.tile([128, 128], F32, tag="feats")
                        nc.vector.tensor_scalar_mul(tmp[:, :], sr_b, b_ap)
                        nc.vector.scalar_tensor_tensor(
                            feats[:, :], cr_b, a_ap, tmp[:, :], op0=mult, op1=add)

                        fh = fpool.tile([128, 128], BF16, tag="fh")
                        fl = fpool.tile([128, 128], BF16, tag="fl")
                        fd = fpool.tile([128, 128], F32, tag="fd")
                        nc.gpsimd.tensor_copy(out=fh[:, :], in_=feats[:, :])
                        nc.vector.tensor_tensor(
                            out=fd[:, :], in0=feats[:, :], in1=fh[:, :], op=sub)
                        nc.gpsimd.tensor_copy(out=fl[:, :], in_=fd[:, :])

                        p = ppool2.tile([128, D], F32, tag="p2")
                        nc.tensor.matmul(p[:, :], fh[:, :], wh_s[:, :],
                                         start=True, stop=False)
                        nc.tensor.matmul(p[:, :], fh[:, :], wl_s[:, :],
                                         start=False, stop=False)
                        nc.tensor.matmul(p[:, :], fl[:, :], wh_s[:, :],
                                         start=False, stop=True)

                        ot = opool.tile([128, D], F32, tag="ot")
                        nc.scalar.activation(ot[:, :], p[:, :], Copy,
                                             bias=0.0, scale=m_ap)
                        nc.sync.dma_start(
                            out=out[t * 128:(t + 1) * 128, :], in_=ot[:, :])

            def body_v3(_iv=None):
                # Engine-balanced: ACT does the SrT*B multiply and the j0
                # PSUM drain, DVE does the combine/sub and the j1 drain,
                # GpSimd does the two f32->bf16 rounding copies, PE runs the
                # 6 bf16 matmuls (the per-tile floor).
                for b in range(BPC):
                    cr_b = crt_s[:, b * 128:(b + 1) * 128]
                    sr_b = srt_s[:, b * 128:(b + 1) * 128]
                    for q in range(QT):
                        t = b * QT + q
                        a_ap = ab_s[:, b * 2 * QT + q: b * 2 * QT + q + 1]
                        b_ap = ab_s[:, b * 2 * QT + QT + q: b * 2 * QT + QT + q + 1]
                        m_ap = msk_s[:, t:t + 1]

                        tmp = fpool.tile([128, 128], F32, tag="tmp")
                        feats = fpool.tile([128, 128], F32, tag="feats")
                        nc.scalar.activation(tmp[:, :], sr_b, Copy,
                                             bias=0.0, scale=b_ap)
                        nc.vector.scalar_tensor_tensor(
                            feats[:, :], cr_b, a_ap, tmp[:, :], op0=mult, op1=add)

                        fh = fpool.tile([128, 128], BF16, tag="fh")
                        fl = fpool.tile([128, 128], BF16, tag="fl")
                        fd = fpool.tile([128, 128], F32, tag="fd")
                        nc.gpsimd.tensor_copy(out=fh[:, :], in_=feats[:, :])
                        nc.vector.tensor_tensor(
                            out=fd[:, :], in0=feats[:, :], in1=fh[:, :], op=sub)
                        nc.gpsimd.tensor_copy(out=fl[:, :], in_=fd[:, :])

                        ot = opool.tile([128, D], F32, tag="ot")
                        for j in range(2):
                            p = ppool.tile([128, 512], F32, tag="p")
                            wslice = slice(512 * j, 512 * (j + 1))
                            nc.tensor.matmul(p[:, :], fh[:, :], wh_s[:, wslice],
                                             start=True, stop=False)
                            nc.tensor.matmul(p[:, :], fh[:, :], wl_s[:, wslice],
                                             start=False, stop=False)
                            nc.tensor.matmul(p[:, :], fl[:, :], wh_s[:, wslice],
                                             start=False, stop=True)
                            if j == 0:
                                nc.scalar.activation(ot[:, wslice], p[:, :],
                                                     Copy, bias=0.0, scale=m_ap)
                            else:
                                nc.vector.tensor_scalar_mul(
                                    ot[:, wslice], p[:, :], m_ap)
                        nc.sync.dma_start(
                            out=out[t * 128:(t + 1) * 128, :], in_=ot[:, :])

            def body_v4(_iv=None):
                # Like bf16split but the 3-matmul groups land in a 2-bank
                # [128,1024] PSUM tile (512-col matmuls into each half) so
                # the masked drain is ONE 1024-wide ACT op; ACT also does the
                # fh rounding copy to relieve DVE. GpSimd stays idle (slower
                # than modeled on real silicon).
                for b in range(BPC):
                    cr_b = crt_s[:, b * 128:(b + 1) * 128]
                    sr_b = srt_s[:, b * 128:(b + 1) * 128]
                    for q in range(QT):
                        t = b * QT + q
                        a_ap = ab_s[:, b * 2 * QT + q: b * 2 * QT + q + 1]
                        b_ap = ab_s[:, b * 2 * QT + QT + q: b * 2 * QT + QT + q + 1]
                        m_ap = msk_s[:, t:t + 1]

                        tmp = fpool.tile([128, 128], F32, tag="tmp")
                        feats = fpool.tile([128, 128], F32, tag="feats")
                        nc.vector.tensor_scalar_mul(tmp[:, :], sr_b, b_ap)
                        nc.vector.scalar_tensor_tensor(
                            feats[:, :], cr_b, a_ap, tmp[:, :], op0=mult, op1=add)

                        fh = fpool.tile([128, 128], BF16, tag="fh")
                        fl = fpool.tile([128, 128], BF16, tag="fl")
                        fd = fpool.tile([128, 128], F32, tag="fd")
                        nc.scalar.activation(fh[:, :], feats[:, :], Copy,
                                             bias=0.0, scale=1.0)
                        nc.vector.tensor_tensor(
                            out=fd[:, :], in0=feats[:, :], in1=fh[:, :], op=sub)
                        nc.vector.tensor_copy(out=fl[:, :], in_=fd[:, :])

                        p = ppool2.tile([128, D], F32, tag="p2")
                        for j in range(2):
                            ps_ap = p[:, 512 * j:512 * (j + 1)]
                            wslice = slice(512 * j, 512 * (j + 1))
                            nc.tensor.matmul(ps_ap, fh[:, :], wh_s[:, wslice],
                                             start=True, stop=False)
                            nc.tensor.matmul(ps_ap, fh[:, :], wl_s[:, wslice],
                                             start=False, stop=False)
                            nc.tensor.matmul(ps_ap, fl[:, :], wh_s[:, wslice],
                                             start=False, stop=True)

                        ot = opool.tile([128, D], F32, tag="ot")
                        nc.scalar.activation(ot[:, :], p[:, :], Copy,
                                             bias=0.0, scale=m_ap)
                        nc.sync.dma_start(
                            out=out[t * 128:(t + 1) * 128, :], in_=ot[:, :])

            def body_v5(_iv=None):
                for s in range(g_slots):
                    cr_b = crts_s[:, s * 128:(s + 1) * 128]
                    sr_b = srts_s[:, s * 128:(s + 1) * 128]
                    a_ap = ab_s[:, s:s + 1]
                    b_ap = ab_s[:, g_slots + s: g_slots + s + 1]
                    m_ap = msk_s[:, s:s + 1]

                    tmp = fpool.tile([128, 128], F32, tag="tmp")
                    feats = fpool.tile([128, 128], F32, tag="feats")
                    nc.vector.tensor_scalar_mul(tmp[:, :], sr_b, b_ap)
                    nc.vector.scalar_tensor_tensor(
                        feats[:, :], cr_b, a_ap, tmp[:, :], op0=mult, op1=add)

                    fh = fpool.tile([128, 128], BF16, tag="fh")
                    fl = fpool.tile([128, 128], BF16, tag="fl")
                    fd = fpool.tile([128, 128], F32, tag="fd")
                    nc.vector.tensor_copy(out=fh[:, :], in_=feats[:, :])
                    nc.vector.tensor_tensor(
                        out=fd[:, :], in0=feats[:, :], in1=fh[:, :], op=sub)
                    nc.vector.tensor_copy(out=fl[:, :], in_=fd[:, :])

                    ot = opool.tile([128, D], F32, tag="ot")
                    for j in range(2):
                        p = ppool.tile([128, 512], F32, tag="p")
                        wslice = slice(512 * j, 512 * (j + 1))
                        nc.tensor.matmul(p[:, :], fh[:, :], wh_s[:, wslice],
                                         start=True, stop=False)
                        nc.tensor.matmul(p[:, :], fh[:, :], wl_s[:, wslice],
                                         start=False, stop=False)
                        nc.tensor.matmul(p[:, :], fl[:, :], wh_s[:, wslice],
                                         start=False, stop=True)
                        nc.scalar.activation(ot[:, wslice], p[:, :], Copy,
                                             bias=0.0, scale=m_ap)
                    nc.sync.dma_start(
                        out=out[s * 128:(s + 1) * 128, :], in_=ot[:, :])

            def body(_iv=None):
                if mm_mode == "v5":
                    body_v5(_iv)
                    return
                if mm_mode == "v4":
                    body_v4(_iv)
                    return
                if mm_mode == "v3":
                    body_v3(_iv)
                    return
                if mm_mode == "v2":
                    body_v2(_iv)
                    return
                if mm_mode == "dmaonly":
                    for t in range(NT):
                        nc.sync.dma_start(
                            out=out[t * 128:(t + 1) * 128, :], in_=zt[:, :])
                    return
                for b in range(BPC):
                    cr_b = crt_s[:, b * 128:(b + 1) * 128]
                    sr_b = srt_s[:, b * 128:(b + 1) * 128]
                    for q in range(QT):
                        t = b * QT + q
                        a_ap = ab_s[:, b * 2 * QT + q: b * 2 * QT + q + 1]
                        b_ap = ab_s[:, b * 2 * QT + QT + q: b * 2 * QT + QT + q + 1]
                        m_ap = msk_s[:, t:t + 1]

                        tmp = fpool.tile([128, 128], F32, tag="tmp")
                        feats = fpool.tile([128, 128], F32, tag="feats")
                        nc.vector.tensor_scalar_mul(tmp[:, :], sr_b, b_ap)
                        nc.vector.scalar_tensor_tensor(
                            feats[:, :], cr_b, a_ap, tmp[:, :], op0=mult, op1=add)

                        if mm_mode in ("bf16split", "nocopy", "nodma", "bal"):
                            eng = nc.gpsimd if mm_mode == "bal" else nc.vector
                            fh = fpool.tile([128, 128], BF16, tag="fh")
                            fl = fpool.tile([128, 128], BF16, tag="fl")
                            fd = fpool.tile([128, 128], F32, tag="fd")
                            eng.tensor_copy(out=fh[:, :], in_=feats[:, :])
                            eng.tensor_tensor(
                                out=fd[:, :], in0=feats[:, :], in1=fh[:, :], op=sub)
                            eng.tensor_copy(out=fl[:, :], in_=fd[:, :])

                        ot = zt if mm_mode == "nocopy" else opool.tile(
                            [128, D], F32, tag="ot")
                        for j in range(2):
                            p = ppool.tile([128, 512], F32, tag="p")
                            wslice = slice(512 * j, 512 * (j + 1))
                            if mm_mode in ("bf16split", "nocopy", "nodma", "bal"):
                                nc.tensor.matmul(
                                    p[:, :], fh[:, :], wh_s[:, wslice],
                                    start=True, stop=False)
                                nc.tensor.matmul(
                                    p[:, :], fh[:, :], wl_s[:, wslice],
                                    start=False, stop=False)
                                nc.tensor.matmul(
                                    p[:, :], fl[:, :], wh_s[:, wslice],
                                    start=False, stop=True)
                            else:
                                nc.tensor.matmul(
                                    p[:, :], feats[:, :], wt_s[:, wslice],
                                    start=True, stop=True)
                            if mm_mode == "bal" and j == 1:
                                nc.vector.tensor_scalar_mul(
                                    ot[:, wslice], p[:, :], m_ap)
                            elif mm_mode != "nocopy":
                                nc.scalar.activation(
                                    ot[:, wslice], p[:, :], Copy,
                                    bias=0.0, scale=m_ap)
                        if mm_mode != "nodma":
                            src_t = zt if mm_mode == "nocopy" else ot
                            nc.sync.dma_start(
                                out=out[t * 128:(t + 1) * 128, :], in_=src_t[:, :])

            if repeat == 1:
                body()
            else:
                with tc.For_i(0, repeat, 1):
                    body()

    _legalize_waits(nc)
    return nc


def _host_tables(lengths):
    """Per-batch f32 tables: CrT/SrT [128,128], A/B [128,QT], M [128,QT]."""
    k = np.arange(1, K + 1, dtype=np.float64)
    crt = np.empty((B, 128, 128), np.float32)
    srt = np.empty((B, 128, 128), np.float32)
    ab = np.empty((B, 128, 2 * QT), np.float32)
    msk = np.empty((B, 128, QT), np.float32)
    r = np.arange(128, dtype=np.float64)
    q128 = 128.0 * np.arange(QT, dtype=np.float64)
    p = np.arange(128)
    for bi in range(B):
        L = float(lengths[bi])
        d = 2.0 * math.pi * k / L                      # [K]
        cr = np.cos(d[:, None] * r[None, :])           # [K,128]
        sr = np.sin(d[:, None] * r[None, :])
        crt[bi] = np.repeat(cr, 2, axis=0)
        srt[bi] = np.repeat(sr, 2, axis=0)
        cq = np.cos(d[:, None] * q128[None, :])        # [K,QT]
        sq = np.sin(d[:, None] * q128[None, :])
        a = np.empty((128, QT), np.float64)
        bb = np.empty((128, QT), np.float64)
        a[0::2] = cq
        a[1::2] = sq
        bb[0::2] = -sq
        bb[1::2] = cq
        ab[bi, :, :QT] = a
        ab[bi, :, QT:] = bb
        msk[bi] = ((q128[None, :] + p[:, None]) < L)
    return crt, srt, ab, msk


def _prepare_in_maps(lengths, W):
    wt = np.ascontiguousarray(W.T * AMP).astype(np.float32)   # [128, D]
    crt, srt, ab, msk = _host_tables(lengths)
    in_maps = []
    for c in range(N_CORES):
        b0, b1 = BPC * c, BPC * c + 1
        in_maps.append({
            "wt": wt,
            "crt": np.ascontiguousarray(np.concatenate([crt[b0], crt[b1]], axis=1)),
            "srt": np.ascontiguousarray(np.concatenate([srt[b0], srt[b1]], axis=1)),
            "ab": np.ascontiguousarray(np.concatenate([ab[b0], ab[b1]], axis=1)),
            "msk": np.ascontiguousarray(np.concatenate([msk[b0], msk[b1]], axis=1)),
        })
    return in_maps


def _prepare_v5(lengths, W):
    """Evenly distribute the ~sum(ceil(L/128)) unmasked tiles over cores.
    Returns (in_maps, slot_lists, G)."""
    wt = np.ascontiguousarray(W.T * AMP).astype(np.float32)
    crt, srt, ab, msk = _host_tables(lengths)
    tiles = [(b, q) for b in range(B)
             for q in range((int(lengths[b]) + 127) // 128)]
    T = len(tiles)
    G = (T + N_CORES - 1) // N_CORES
    slot_lists, in_maps = [], []
    for c in range(N_CORES):
        lst = tiles[c * G:(c + 1) * G]
        real = len(lst)
        if real == 0:
            lst = [tiles[-1]] * G
        elif real < G:
            lst = lst + [lst[-1]] * (G - real)
        slot_lists.append((lst, real))
        crt_sl = np.empty((128, G * 128), np.float32)
        srt_sl = np.empty((128, G * 128), np.float32)
        ab_sl = np.empty((128, 2 * G), np.float32)
        msk_sl = np.empty((128, G), np.float32)
        for s, (b, q) in enumerate(lst):
            crt_sl[:, s * 128:(s + 1) * 128] = crt[b]
            srt_sl[:, s * 128:(s + 1) * 128] = srt[b]
            ab_sl[:, s] = ab[b][:, q]
            ab_sl[:, G + s] = ab[b][:, QT + q]
            msk_sl[:, s] = msk[b][:, q]
        in_maps.append({"wt": wt, "crt": crt_sl, "srt": srt_sl,
                        "ab": ab_sl, "msk": msk_sl})
    return in_maps, slot_lists, G


def kernel(lengths, W, N_max):
    global LAST_RESULT
    lengths = np.asarray(lengths)
    W = np.asarray(W, dtype=np.float32)
    n_max = int(N_max)
    assert n_max == N_MAX and W.shape == (D, 2 * K) and lengths.shape == (B,)

    if MM_MODE == "v5":
        in_maps, slot_lists, G = _prepare_v5(lengths, W)
        nc = _build_program(repeat=REPEAT, mm_mode="v5", g_slots=G)
        res = run_bass_kernel_spmd(nc, in_maps, core_ids=list(range(N_CORES)))
        LAST_RESULT = res
        pos_emb = np.zeros((B, N_MAX, D), np.float32)
        for c in range(N_CORES):
            o = res.results[c]["out"]
            lst, real = slot_lists[c]
            for s in range(real):
                b, q = lst[s]
                pos_emb[b, q * 128:(q + 1) * 128] = o[s * 128:(s + 1) * 128]
        mask = np.arange(N_MAX)[None, :] < np.asarray(lengths).astype(np.int64)[:, None]
        return pos_emb, mask

    in_maps = _prepare_in_maps(lengths, W)
    nc = _build_program(repeat=REPEAT, mm_mode=MM_MODE)

    res = run_bass_kernel_spmd(nc, in_maps, core_ids=list(range(N_CORES)))
    LAST_RESULT = res

    pos_emb = np.empty((B, N_MAX, D), np.float32)
    for c in range(N_CORES):
        pos_emb[BPC * c:BPC * (c + 1)] = res.results[c]["out"].reshape(BPC, N_MAX, D)

    mask = np.arange(N_MAX)[None, :] < np.asarray(lengths).astype(np.int64)[:, None]
    return pos_emb, mask
